# revision 8
# baseline (speedup 1.0000x reference)
"""DRCN forward pass as a Bass/Tile kernel for Trainium2.

Data-parallel over batch: the full batch is split across 8 NeuronCores.
On-chip sequence tensors use a transposed batch-major layout
[feature(partition), b*L + t(free)] so matmuls contract over features on
the partition dim and the LSTM recurrence keeps state as [H, B] tiles.
"""

import numpy as np

_H = 100


def _build_program(B, L, V, E, H=_H):
    """Build the single-core Bass program (SPMD across cores; data differs
    only via inputs). B = per-core batch."""
    import concourse.bass as bass
    import concourse.bacc as bacc
    import concourse.tile as tile
    from concourse import mybir
    from concourse.masks import make_identity

    f32 = mybir.dt.float32
    f32r = mybir.dt.float32r
    i32 = mybir.dt.int32
    AF = mybir.ActivationFunctionType
    ALU = mybir.AluOpType
    AX = mybir.AxisListType

    NT = B * L                       # tokens per core, col = b*L + t
    NC = (L + 127) // 128            # 128-chunks per sequence
    G4 = 4 * B                       # gate block width in psum
    assert G4 <= 128
    W_AE = min(512, NT)              # AE column block
    assert W_AE % L == 0
    NBLK = (NT + W_AE - 1) // W_AE
    NTILE = (NT + 127) // 128        # embed gather tiles

    nc = bacc.Bacc("TRN2", target_bir_lowering=False, debug=False)

    # ---------------- DRAM I/O ----------------
    idx = {s: nc.dram_tensor(f"idx_{s}", [NT, 1], i32, kind="ExternalInput")
           for s in "pq"}
    embed = nc.dram_tensor("embed", [V, E], f32, kind="ExternalInput")
    n_ins = [E, 5 * H, 7 * H, 9 * H, 11 * H]
    Wih, Whh, bias = {}, {}, {}
    for l in range(5):
        Wih[l] = nc.dram_tensor(f"Wih{l}", [n_ins[l], 4 * H], f32, kind="ExternalInput")
        Whh[l] = nc.dram_tensor(f"Whh{l}", [H, 4 * H], f32, kind="ExternalInput")
        bias[l] = nc.dram_tensor(f"bias{l}", [4 * H, 1], f32, kind="ExternalInput")
    a1w1 = nc.dram_tensor("a1w1", [9 * H, 200], f32, kind="ExternalInput")
    a1b1 = nc.dram_tensor("a1b1", [200, 1], f32, kind="ExternalInput")
    a1w2 = nc.dram_tensor("a1w2", [200, 9 * H], f32, kind="ExternalInput")
    a1b2 = nc.dram_tensor("a1b2", [9 * H, 1], f32, kind="ExternalInput")
    a2w1 = nc.dram_tensor("a2w1", [13 * H, 200], f32, kind="ExternalInput")
    a2b1 = nc.dram_tensor("a2b1", [200, 1], f32, kind="ExternalInput")
    a2w2 = nc.dram_tensor("a2w2", [200, 13 * H], f32, kind="ExternalInput")
    a2b2 = nc.dram_tensor("a2b2", [13 * H, 1], f32, kind="ExternalInput")
    dw1 = nc.dram_tensor("dw1", [65 * H, 1000], f32, kind="ExternalInput")
    db1 = nc.dram_tensor("db1", [1000, 1], f32, kind="ExternalInput")
    dw2 = nc.dram_tensor("dw2", [1000, 1000], f32, kind="ExternalInput")
    db2 = nc.dram_tensor("db2", [1000, 1], f32, kind="ExternalInput")
    dw3 = nc.dram_tensor("dw3", [1000, 2], f32, kind="ExternalInput")
    db3 = nc.dram_tensor("db3", [2, 1], f32, kind="ExternalInput")
    out_d = nc.dram_tensor("out", [B, 2], f32, kind="ExternalOutput")

    # internal DRAM sequence tensors (feature-major, cols = b*L+t)
    xembT = {s: nc.dram_tensor(f"xembT_{s}", [E, NT], f32r) for s in "pq"}
    hT_d = {(l, s): nc.dram_tensor(f"hT{l}_{s}", [H, NT], f32)
            for l in range(5) for s in "pq"}
    aT_d = {(l, s): nc.dram_tensor(f"aT{l}_{s}", [H, NT], f32r)
            for l in range(5) for s in "pq"}
    x4T_d = {s: nc.dram_tensor(f"x4T_{s}", [9 * H, NT], f32r) for s in "pq"}

    # torch gate order in weights: i,f,g,o ; psum block order: f,i,o,g
    GSL = [(100, 200), (0, 100), (300, 400), (200, 300)]

    def rmm(out, lhsT, rhs, **kw):
        """matmul on tf32 (float32r) operands - full-rate when N>=256."""
        nc.tensor.matmul(out, lhsT, rhs, **kw)

    with tile.TileContext(nc) as tc:
        with (
            tc.tile_pool(name="ps", bufs=1, space="PSUM") as ps,
            tc.tile_pool(name="sb", bufs=1) as sb,
        ):
            # PSUM: 8 banks total: psA(3) + psB(3) + psC(2)
            def psA(shape):
                return ps.tile(shape, f32, tag="psA", bufs=3,
                               name=f"psA{nc.next_id()}")

            def psB(shape):
                return ps.tile(shape, f32, tag="psB", bufs=3,
                               name=f"psB{nc.next_id()}")

            def psC(shape):
                return ps.tile(shape, f32, tag="psC", bufs=2,
                               name=f"psC{nc.next_id()}")

            def sbt(shape, tag, bufs=1, dt=f32):
                return sb.tile(shape, dt, tag=tag, bufs=bufs,
                               name=f"{tag}{nc.next_id()}")

            # ---------- constants ----------
            ident = sbt([128, 128], "ident")
            make_identity(nc, ident[:])
            ones = sbt([128, 128], "ones")
            nc.vector.memset(ones[:], 1.0)
            ones_r = sbt([128, 128], "ones_r", dt=f32r)
            nc.vector.tensor_copy(ones_r[:], ones[:])

            def load_bias_col(dram, k):
                t = sbt([100, 1], f"bc_{dram.name}_{k}")
                nc.sync.dma_start(t[:], dram[k * 100:(k + 1) * 100, :])
                return t

            # ---------- embed: t=0 gather -> x0 comps ----------
            x0 = {s: [] for s in "pq"}   # list of [100,B] tiles per tensor
            idx_r = {s: idx[s][:].rearrange("(b l) o -> b l o", l=L)
                     for s in "pq"}
            for s in "pq":
                it = sbt([B, 1], f"idx0_{s}", dt=i32)
                nc.sync.dma_start(it[:], idx_r[s][:, 0, :])
                g = sbt([B, E], f"g0_{s}")
                nc.gpsimd.indirect_dma_start(
                    out=g[:], out_offset=None, in_=embed[:],
                    in_offset=bass.IndirectOffsetOnAxis(ap=it[:, :1], axis=0))
                for k in range(E // 100):
                    pt = psB([100, B])
                    nc.tensor.transpose(pt[:], g[:, k * 100:(k + 1) * 100],
                                        ident[:B, :B])
                    xt = sbt([100, B], f"x0_{s}_{k}")
                    nc.scalar.copy(xt[:], pt[:])
                    x0[s].append(xt)

            # ---------- embed: bulk gather -> xembT DRAM ----------
            for s in "pq":
                for r in range(NTILE):
                    rows_n = min(128, NT - r * 128)
                    it = sbt([128, 1], "idx_blk", bufs=2, dt=i32)
                    nc.sync.dma_start(it[:rows_n, :],
                                      idx[s][r * 128:r * 128 + rows_n, :])
                    g = sbt([128, E], "g_blk", bufs=2)
                    nc.gpsimd.indirect_dma_start(
                        out=g[:rows_n, :], out_offset=None, in_=embed[:],
                        in_offset=bass.IndirectOffsetOnAxis(ap=it[:rows_n, :1],
                                                            axis=0))
                    for k in range(E // 100):
                        pt = psB([100, 128])
                        nc.tensor.transpose(pt[:, :rows_n],
                                            g[:rows_n, k * 100:(k + 1) * 100],
                                            ident[:rows_n, :rows_n])
                        st = sbt([100, 128], "gt_blk", bufs=2, dt=f32r)
                        nc.scalar.copy(st[:, :rows_n], pt[:, :rows_n])
                        nc.sync.dma_start(
                            xembT[s][k * 100:(k + 1) * 100,
                                     r * 128:r * 128 + rows_n],
                            st[:, :rows_n])

            # ---------- helpers ----------
            def build_pre(x0_list, wih_tiles, bias_cols, qtag):
                """preTT [4B, 100] for one scan."""
                pp = psA([100, G4])
                for g in range(4):
                    gs = GSL[g]
                    for k, xt in enumerate(x0_list):
                        nc.tensor.matmul(
                            pp[:, g * B:(g + 1) * B],
                            wih_tiles[k][:, gs[0]:gs[1]], xt[:],
                            start=(k == 0), stop=(k == len(x0_list) - 1))
                preT = sbt([100, G4], "preT", bufs=3)
                for g in range(4):
                    nc.scalar.activation(preT[:, g * B:(g + 1) * B],
                                         pp[:, g * B:(g + 1) * B],
                                         AF.Identity, bias=bias_cols[g][:])
                pt = psB([G4, 100])
                nc.tensor.transpose(pt[:], preT[:], ident[:100, :100])
                preTT = sbt([G4, 100], "preTT", bufs=3)
                nc.scalar.copy(preTT[:], pt[:])
                return preTT

            def scan(preTT, whh_tiles, h_init_ap, cbuf, hsT):
                """LSTM scan; writes hsT cols (b*L+t); returns final-h AP.
                cbuf [100, 2B]; cbuf[:,0:B] must hold c0 on entry."""
                hsT_r = hsT[:].rearrange("h (b l) -> h b l", l=L)
                h_prev = h_init_ap
                for t in range(L):
                    pg = psA([100, G4])
                    nc.tensor.matmul(pg[:], preTT[:], ident[:G4, :G4],
                                     start=True, stop=False)
                    for g in range(4):
                        nc.tensor.matmul(pg[:, g * B:(g + 1) * B],
                                         whh_tiles[g][:], h_prev,
                                         start=False, stop=(g == 3))
                    sfio = sbt([100, 3 * B], "sfio", bufs=3)
                    nc.scalar.activation(sfio[:], pg[:, 0:3 * B], AF.Sigmoid)
                    nc.scalar.activation(cbuf[:, B:2 * B], pg[:, 3 * B:G4],
                                         AF.Tanh)
                    z = sbt([100, 2 * B], "z", bufs=3)
                    nc.vector.tensor_mul(z[:], cbuf[:], sfio[:, 0:2 * B])
                    nc.vector.tensor_add(cbuf[:, 0:B], z[:, 0:B], z[:, B:2 * B])
                    tct = sbt([100, B], "tct", bufs=3)
                    nc.scalar.activation(tct[:], cbuf[:, 0:B], AF.Tanh)
                    h_slice = hsT_r[:, :, t]
                    nc.vector.tensor_mul(h_slice, sfio[:, 2 * B:3 * B], tct[:])
                    h_prev = h_slice
                return h_prev

            def attention(l, hsT_p, hsT_q):
                """Writes aT DRAM for both sides; returns (a0_p, a0_q)."""
                a0 = {s: sbt([100, B], f"a0_{l}_{s}") for s in "pq"}
                hsT = {"p": hsT_p, "q": hsT_q}
                for b in range(B):
                    pb_cols = hsT_p[:, b * L:(b + 1) * L]
                    qb_cols = hsT_q[:, b * L:(b + 1) * L]
                    # row tiles + q-side sq-norm accumulators for this b
                    rows = {"p": [], "q": []}
                    rnq = []
                    for s in "pq":
                        for c in range(NC):
                            cl = min(128, L - c * 128)
                            src = hsT[s][:, b * L + c * 128:b * L + c * 128 + cl]
                            pt = psB([128, 100])
                            nc.tensor.transpose(pt[:cl, :], src,
                                                ident[:100, :100])
                            rt = sbt([128, 100], "rows", bufs=4 * NC + 4,
                                      dt=f32r)
                            nc.scalar.copy(rt[:cl, :], pt[:cl, :])
                            rows[s].append(rt)
                            if s == "q":
                                scr = sbt([128, 100], "sq_scr", bufs=2)
                                nq = sbt([128, 1], "rnq", bufs=2 * NC + 2)
                                nc.scalar.activation(scr[:cl, :], rt[:cl, :],
                                                     AF.Square,
                                                     accum_out=nq[:cl, :])
                                nc.scalar.activation(nq[:cl, :], nq[:cl, :],
                                                     AF.Sqrt)
                                nc.vector.tensor_scalar_max(nq[:cl, :],
                                                            nq[:cl, :], 1e-4)
                                nc.vector.reciprocal(nq[:cl, :], nq[:cl, :])
                                rnq.append(nq)
                    # p-side norms as a row: pn2 = ones_col.T @ (pb^2)
                    sqp = sbt([100, L], "sqp", bufs=2, dt=f32r)
                    nc.scalar.activation(sqp[:], pb_cols, AF.Square)
                    n2 = psC([1, L])
                    rmm(n2[:], ones_r[0:100, 0:1], sqp[:], start=True, stop=True)
                    rnp = sbt([1, L], "rnp", bufs=2, dt=f32r)
                    nc.scalar.activation(rnp[:], n2[:], AF.Sqrt)
                    nc.vector.tensor_scalar_max(rnp[:], rnp[:], 1e-4)
                    with nc.allow_low_precision(reason="tf32 attn scale"):
                        nc.vector.reciprocal(rnp[:], rnp[:])
                    bcp = psB([100, L])
                    rmm(bcp[:], ones_r[0:1, 0:100], rnp[:],
                        start=True, stop=True)
                    phatT = sbt([100, L], "phatT", bufs=2, dt=f32r)
                    nc.vector.tensor_mul(phatT[:], pb_cols, bcp[:])
                    # ET_c = exp((qhat_j . phat_i)) in [j, i] layout
                    ET, EThat = [], []
                    for c in range(NC):
                        cl = min(128, L - c * 128)
                        pe = psA([128, L])
                        qh = sbt([100, 128], "qh", bufs=3, dt=f32r)
                        nc.vector.tensor_copy(qh[:, :cl],
                                              qb_cols[:, c * 128:c * 128 + cl])
                        rmm(pe[:cl, :], qh[:, :cl], phatT[:],
                            start=True, stop=True)
                        et = sbt([128, L], "et", bufs=2, dt=f32r)
                        cq = sbt([128, 1], "cq", bufs=3)
                        nc.scalar.activation(et[:cl, :], pe[:cl, :], AF.Exp,
                                             scale=rnq[c][:cl, :],
                                             accum_out=cq[:cl, :])
                        rq = sbt([128, 1], "rq", bufs=3)
                        nc.vector.reciprocal(rq[:cl, :], cq[:cl, :])
                        eh = sbt([128, L], "eth", bufs=2, dt=f32r)
                        nc.vector.tensor_scalar_mul(eh[:cl, :], et[:cl, :],
                                                    rq[:cl, :])
                        ET.append(et)
                        EThat.append(eh)
                    # aT_q = sum_j pb[j,:]^T EThat[j,i]
                    pq_ = psB([100, L])
                    for c in range(NC):
                        cl = min(128, L - c * 128)
                        rmm(pq_[:], rows["p"][c][:cl, :], EThat[c][:cl, :],
                            start=(c == 0), stop=(c == NC - 1))
                    aqs = sbt([100, L], "aqs", bufs=2, dt=f32r)
                    nc.vector.tensor_copy(aqs[:], pq_[:])
                    nc.sync.dma_start(aT_d[l, "q"][:, b * L:(b + 1) * L], aqs[:])
                    nc.vector.tensor_copy(a0["q"][:, b:b + 1], aqs[:, 0:1])
                    # aT_p (unnormalized) + row-sum normalization
                    pp_ = psB([100, L])
                    for c in range(NC):
                        cl = min(128, L - c * 128)
                        rmm(pp_[:], rows["q"][c][:cl, :], ET[c][:cl, :],
                            start=(c == 0), stop=(c == NC - 1))
                    rp = psC([1, L])
                    for c in range(NC):
                        cl = min(128, L - c * 128)
                        rmm(rp[:], ones_r[0:cl, 0:1], ET[c][:cl, :],
                            start=(c == 0), stop=(c == NC - 1))
                    rps = sbt([1, L], "rps", bufs=2, dt=f32r)
                    nc.scalar.copy(rps[:], rp[:])
                    with nc.allow_low_precision(reason="tf32 attn scale"):
                        nc.vector.reciprocal(rps[:], rps[:])
                    rpb = psB([100, L])
                    rmm(rpb[:], ones_r[0:1, 0:100], rps[:],
                        start=True, stop=True)
                    aps = sbt([100, L], "aps", bufs=2, dt=f32r)
                    app = sbt([100, L], "app", bufs=2)
                    nc.scalar.copy(app[:], pp_[:])
                    nc.vector.tensor_mul(aps[:], app[:], rpb[:])
                    nc.sync.dma_start(aT_d[l, "p"][:, b * L:(b + 1) * L], aps[:])
                    nc.vector.tensor_copy(a0["p"][:, b:b + 1], aps[:, 0:1])
                return a0["p"], a0["q"]

            def mini_ae(x0_list, w1t, b1c, w2t, b2c, m1, m2, qtag):
                """AE applied to the t=0 columns only (tf32 matmuls)."""
                x0r = []
                for k, xt in enumerate(x0_list):
                    t = sbt([100, B], "x0r", bufs=11, dt=f32r)
                    nc.vector.tensor_copy(t[:], xt[:])
                    x0r.append(t)
                h1 = []
                for m in range(m1):
                    pp = psA([100, B])
                    for k, xt in enumerate(x0r):
                        nc.tensor.matmul(pp[:], w1t[k][:, m * 100:(m + 1) * 100],
                                         xt[:], start=(k == 0),
                                         stop=(k == len(x0r) - 1))
                    t = sbt([100, B], f"mh1_{qtag}_{m}", dt=f32r)
                    nc.scalar.activation(t[:], pp[:], AF.Tanh, bias=b1c[m][:])
                    h1.append(t)
                out = []
                for m in range(m2):
                    pp = psA([100, B])
                    for k in range(m1):
                        nc.tensor.matmul(pp[:], w2t[k][:, m * 100:(m + 1) * 100],
                                         h1[k][:], start=(k == 0),
                                         stop=(k == m1 - 1))
                    t = sbt([100, B], f"mo_{qtag}_{m}")
                    nc.scalar.activation(t[:], pp[:], AF.Tanh, bias=b2c[m][:])
                    out.append(t)
                return out

            def bulk_ae(comps, w1t, b1c, w2t, b2c, m1, m2, out_dram, vout):
                """Streaming AE over all NT cols. comps: list of (dram, row0).
                vout (list of m2 [100,B] tiles): fused max-over-t instead of
                a DRAM write."""
                K1 = len(comps)
                for blk in range(NBLK):
                    c0 = blk * W_AE
                    w = min(W_AE, NT - c0)
                    pps = [psA([100, W_AE]) for _ in range(m1)]
                    for k, (dr, r0) in enumerate(comps):
                        if dr.dtype == f32r:
                            rhs = sbt([100, W_AE], "ae_in", bufs=4, dt=f32r)
                            nc.sync.dma_start(rhs[:, :w],
                                              dr[r0:r0 + 100, c0:c0 + w])
                        else:
                            tmp = sbt([100, W_AE], "ae_tmp", bufs=2)
                            nc.sync.dma_start(tmp[:, :w],
                                              dr[r0:r0 + 100, c0:c0 + w])
                            rhs = sbt([100, W_AE], "ae_in", bufs=4, dt=f32r)
                            nc.vector.tensor_copy(rhs[:, :w], tmp[:, :w])
                        for m in range(m1):
                            rmm(pps[m][:, :w], w1t[k][:, m * 100:(m + 1) * 100],
                                rhs[:, :w], start=(k == 0), stop=(k == K1 - 1))
                    h1 = []
                    for m in range(m1):
                        t = sbt([100, W_AE], "ae_h1", bufs=m1 + 1, dt=f32r)
                        nc.scalar.activation(t[:, :w], pps[m][:, :w], AF.Tanh,
                                             bias=b1c[m][:])
                        h1.append(t)
                    for m in range(m2):
                        pp = psB([100, W_AE])
                        for k in range(m1):
                            rmm(pp[:, :w], w2t[k][:, m * 100:(m + 1) * 100],
                                h1[k][:, :w], start=(k == 0), stop=(k == m1 - 1))
                        ot = sbt([100, W_AE], "ae_o", bufs=2, dt=f32r)
                        nc.scalar.activation(ot[:, :w], pp[:, :w], AF.Tanh,
                                             bias=b2c[m][:])
                        if vout is None:
                            nc.sync.dma_start(
                                out_dram[m * 100:(m + 1) * 100, c0:c0 + w],
                                ot[:, :w])
                        else:
                            for j in range(w // L):
                                b_idx = (c0 + j * L) // L
                                nc.vector.tensor_reduce(
                                    vout[m][:, b_idx:b_idx + 1],
                                    ot[:, j * L:(j + 1) * L], AX.X, ALU.max)

            # ---------- layer loop ----------
            x0_p = x0_q = None
            x4_0 = None
            for l in range(5):
                n_comp = n_ins[l] // 100
                wih_tiles = []
                for k in range(n_comp):
                    t = sbt([100, 4 * H], "wih", bufs=11)
                    nc.sync.dma_start(t[:], Wih[l][k * 100:(k + 1) * 100, :])
                    wih_tiles.append(t)
                whh_tiles = []
                for g in range(4):
                    gs = GSL[g]
                    t = sbt([100, 100], "whh", bufs=4)
                    nc.sync.dma_start(t[:], Whh[l][:, gs[0]:gs[1]])
                    whh_tiles.append(t)
                bias_cols = [load_bias_col(bias[l], GSL[g][0] // 100)
                             for g in range(4)]

                if l == 0:
                    x0_p, x0_q = x0["p"], x0["q"]
                elif l == 3:
                    x0_p, x0_q = x4_0["p"], x4_0["q"]

                preTT_p = build_pre(x0_p, wih_tiles, bias_cols, f"{l}p")
                preTT_q = build_pre(x0_q, wih_tiles, bias_cols, f"{l}q")

                hsT_p = sbt([100, NT], "hsT", bufs=2)
                hsT_q = sbt([100, NT], "hsT", bufs=2)
                cbuf_p = sbt([100, 2 * B], f"cb_{l}p")
                cbuf_q = sbt([100, 2 * B], f"cb_{l}q")
                nc.vector.memset(cbuf_p[:, 0:B], 1.0)
                hf = scan(preTT_p, whh_tiles, ones[0:100, 0:B], cbuf_p, hsT_p)
                nc.vector.tensor_copy(cbuf_q[:, 0:B], cbuf_p[:, 0:B])
                scan(preTT_q, whh_tiles, hf, cbuf_q, hsT_q)

                nc.sync.dma_start(hT_d[l, "p"][:], hsT_p[:])
                nc.sync.dma_start(hT_d[l, "q"][:], hsT_q[:])

                # h(t=0) for the next layer's x0
                h0t = {}
                if l < 4:
                    for s, hs in (("p", hsT_p), ("q", hsT_q)):
                        t = sbt([100, B], f"h0_{l}{s}")
                        hs_r = hs[:].rearrange("h (b l) -> h b l", l=L)
                        nc.vector.tensor_copy(t[:], hs_r[:, :, 0])
                        h0t[s] = t

                a0_p, a0_q = attention(l, hsT_p, hsT_q)

                if l == 2:
                    # AE1: mini (t=0 cols) + bulk (streamed, to DRAM)
                    w1t, w2t = [], []
                    for k in range(9):
                        tf_ = sbt([100, 200], "aewt", bufs=2)
                        nc.sync.dma_start(tf_[:], a1w1[k * 100:(k + 1) * 100, :])
                        t = sbt([100, 200], "aew1", bufs=13, dt=f32r)
                        nc.vector.tensor_copy(t[:], tf_[:])
                        w1t.append(t)
                    for k in range(2):
                        tf_ = sbt([100, 13 * H], "aewt2", bufs=2)
                        nc.sync.dma_start(tf_[:, :9 * H],
                                          a1w2[k * 100:(k + 1) * 100, :])
                        t = sbt([100, 13 * H], "aew2", bufs=2, dt=f32r)
                        nc.vector.tensor_copy(t[:, :9 * H], tf_[:, :9 * H])
                        w2t.append(t)
                    b1c = [load_bias_col(a1b1, m) for m in range(2)]
                    b2c = [load_bias_col(a1b2, m) for m in range(9)]
                    mini_in = {"p": [h0t["p"], a0_p] + x0_p,
                               "q": [h0t["q"], a0_q] + x0_q}
                    comps_pe = {}
                    for s in "pq":
                        comps_pe[s] = ([(hT_d[2, s], 0), (aT_d[2, s], 0),
                                        (hT_d[1, s], 0), (aT_d[1, s], 0),
                                        (hT_d[0, s], 0), (aT_d[0, s], 0)]
                                       + [(xembT[s], k * 100)
                                          for k in range(E // 100)])
                    x4_0 = {}
                    for s in "pq":
                        x4_0[s] = mini_ae(mini_in[s], w1t, b1c,
                                          [w[:, :9 * H] for w in w2t],
                                          b2c, 2, 9, f"a1{s}")
                        bulk_ae(comps_pe[s], w1t, b1c,
                                [w[:, :9 * H] for w in w2t], b2c, 2, 9,
                                x4T_d[s], None)
                    x0_p = x0_q = None
                elif l < 4:
                    x0_p = [h0t["p"], a0_p] + x0_p
                    x0_q = [h0t["q"], a0_q] + x0_q

            # ---------- AE2 (fused max-over-time) ----------
            w1t, w2t = [], []
            for k in range(13):
                tf_ = sbt([100, 200], "aewt", bufs=2)
                nc.sync.dma_start(tf_[:], a2w1[k * 100:(k + 1) * 100, :])
                t = sbt([100, 200], "aew1", bufs=13, dt=f32r)
                nc.vector.tensor_copy(t[:], tf_[:])
                w1t.append(t)
            for k in range(2):
                tf_ = sbt([100, 13 * H], "aewt2", bufs=2)
                nc.sync.dma_start(tf_[:], a2w2[k * 100:(k + 1) * 100, :])
                t = sbt([100, 13 * H], "aew2", bufs=2, dt=f32r)
                nc.vector.tensor_copy(t[:], tf_[:])
                w2t.append(t)
            b1c = [load_bias_col(a2b1, m) for m in range(2)]
            b2c = [load_bias_col(a2b2, m) for m in range(13)]
            v = {}
            for s in "pq":
                comps = ([(hT_d[4, s], 0), (aT_d[4, s], 0),
                          (hT_d[3, s], 0), (aT_d[3, s], 0)]
                         + [(x4T_d[s], k * 100) for k in range(9)])
                v[s] = [sbt([100, B], f"v_{s}_{m}") for m in range(13)]
                bulk_ae(comps, w1t, b1c, w2t, b2c, 2, 13, None, v[s])

            # ---------- DNN head ----------
            vec = list(v["p"]) + list(v["q"])
            for m in range(13):
                t = sbt([100, B], f"vsum_{m}")
                nc.vector.tensor_add(t[:], v["p"][m][:], v["q"][m][:])
                vec.append(t)
            dif = []
            for m in range(13):
                t = sbt([100, B], f"vdif_{m}")
                nc.vector.tensor_tensor(t[:], v["p"][m][:], v["q"][m][:],
                                        op=ALU.subtract)
                dif.append(t)
            vec += dif
            for m in range(13):
                t = sbt([100, B], f"vabs_{m}")
                nc.scalar.activation(t[:], dif[m][:], AF.Abs)
                vec.append(t)
            assert len(vec) == 65

            db1c = [load_bias_col(db1, m) for m in range(10)]
            db2c = [load_bias_col(db2, m) for m in range(10)]
            h1t = []
            for m in range(10):
                pp = psA([100, B])
                for k in range(65):
                    wt = sbt([100, 100], "dwc", bufs=6)
                    nc.sync.dma_start(wt[:], dw1[k * 100:(k + 1) * 100,
                                                  m * 100:(m + 1) * 100])
                    nc.tensor.matmul(pp[:], wt[:], vec[k][:],
                                     start=(k == 0), stop=(k == 64))
                t = sbt([100, B], f"h1_{m}")
                nc.scalar.activation(t[:], pp[:], AF.Relu, bias=db1c[m][:])
                h1t.append(t)
            h2t = []
            for m in range(10):
                pp = psA([100, B])
                for k in range(10):
                    wt = sbt([100, 100], "dwc", bufs=6)
                    nc.sync.dma_start(wt[:], dw2[k * 100:(k + 1) * 100,
                                                  m * 100:(m + 1) * 100])
                    nc.tensor.matmul(pp[:], wt[:], h1t[k][:],
                                     start=(k == 0), stop=(k == 9))
                t = sbt([100, B], f"h2_{m}")
                nc.scalar.activation(t[:], pp[:], AF.Relu, bias=db2c[m][:])
                h2t.append(t)
            pp = psA([2, B])
            for k in range(10):
                wt = sbt([100, 2], "dw3c", bufs=4)
                nc.sync.dma_start(wt[:], dw3[k * 100:(k + 1) * 100, :])
                nc.tensor.matmul(pp[:], wt[:], h2t[k][:],
                                 start=(k == 0), stop=(k == 9))
            db3c = sbt([2, 1], "db3c")
            nc.sync.dma_start(db3c[:], db3[:])
            lg = sbt([2, B], "lg")
            nc.scalar.activation(lg[:], pp[:], AF.Identity, bias=db3c[:])
            ptl = psB([B, 2])
            nc.tensor.transpose(ptl[:], lg[:], ident[:2, :2])
            lgT = sbt([B, 2], "lgT")
            nc.scalar.copy(lgT[:], ptl[:])
            mx = sbt([B, 1], "mx")
            nc.vector.tensor_reduce(mx[:], lgT[:], AX.X, ALU.max)
            e1 = sbt([B, 2], "e1")
            nc.vector.tensor_scalar(e1[:], lgT[:], mx[:], None, op0=ALU.subtract)
            e2 = sbt([B, 2], "e2")
            sm = sbt([B, 1], "sm")
            nc.scalar.activation(e2[:], e1[:], AF.Exp, accum_out=sm[:])
            nc.vector.reciprocal(sm[:], sm[:])
            fo = sbt([B, 2], "fo")
            nc.vector.tensor_scalar_mul(fo[:], e2[:], sm[:])
            nc.sync.dma_start(out_d[:], fo[:])

    nc.compile()
    return nc


_NC_CACHE = {}


def _get_program(B, L, V, E):
    key = (B, L, V, E)
    if key not in _NC_CACHE:
        _NC_CACHE[key] = _build_program(B, L, V, E)
    return _NC_CACHE[key]


def _make_in_map(B, L, inputs, core):
    f = np.float32
    p = np.asarray(inputs["p"])[core * B:(core + 1) * B]
    q = np.asarray(inputs["q"])[core * B:(core + 1) * B]
    m = {
        "idx_p": np.ascontiguousarray(p.reshape(-1, 1)).astype(np.int32),
        "idx_q": np.ascontiguousarray(q.reshape(-1, 1)).astype(np.int32),
        "embed": np.asarray(inputs["embed"], dtype=f),
    }
    for l in range(5):
        m[f"Wih{l}"] = np.asarray(inputs[f"Wih{l + 1}"], dtype=f)
        m[f"Whh{l}"] = np.asarray(inputs[f"Whh{l + 1}"], dtype=f)
        m[f"bias{l}"] = np.ascontiguousarray(
            (np.asarray(inputs[f"bih{l + 1}"], dtype=f)
             + np.asarray(inputs[f"bhh{l + 1}"], dtype=f)).reshape(-1, 1))
    for k in ("a1w1", "a1w2", "a2w1", "a2w2", "dw1", "dw2", "dw3"):
        m[k] = np.asarray(inputs[k], dtype=f)
    for k in ("a1b1", "a1b2", "a2b1", "a2b2", "db1", "db2", "db3"):
        m[k] = np.ascontiguousarray(
            np.asarray(inputs[k], dtype=f).reshape(-1, 1))
    return m


def kernel(**inputs):
    from concourse.bass_utils import run_bass_kernel_spmd

    p = np.asarray(inputs["p"])
    B_full, L = p.shape
    V, E = np.asarray(inputs["embed"]).shape
    n_cores = 8
    B = B_full // n_cores
    nc = _get_program(B, L, V, E)
    in_maps = [_make_in_map(B, L, inputs, c) for c in range(n_cores)]
    res = run_bass_kernel_spmd(nc, in_maps, list(range(n_cores)))
    outs = [res.results[c]["out"] for c in range(n_cores)]
    return np.concatenate(outs, axis=0).astype(np.float32)
